# revision 48
# baseline (speedup 1.0000x reference)
"""Trainium2 Bass kernel for a sliding-window self-attention block.

The reference network applies softmax over a singleton axis, so the attention
weights are identically 1.0 and the whole module reduces to:

    h   = relu((x + pos_enc) @ W1 + b1)            # [B, S, 64]
    p   = h @ (Wv @ W2)                            # [B, S, 2]
    out = sliding_window_sum(p, +-8, zero-pad) + b2
    weights = ones([B, S, 1, 17])

Sharding: data-parallel over batch, 2 batches per core on 8 NeuronCores.

Per-core dataflow (batches b0/b1 processed jointly on 128 partitions):
  - Inputs arrive feature-major ([B, I, S]; the host folds the transpose
    into the shard copy it makes anyway), so partition (b, f) tiles load
    with 16 KB-contiguous rows and no on-chip transpose is needed.
  - DVE/GpSimd adds -> h0^T in f32r, directly the matmul moving operand.
  - PE warmup matmuls run during the load phase to release the HAM clock
    gate before real work arrives.
  - One block-diagonal f32r matmul per 512-col chunk applies W1 to both
    batches; ACT relu(+b1); second block-diag matmul applies Wc = Wv@W2.
  - p^T chunks scatter (un-permuting seq) into a zero-padded [4, 4240]
    buffer; 3 SBUF DMAs re-partition it into a [(b,f,c), 144] halo
    layout; 5 log-tree DVE adds compute the 17-wide window sum; ACT adds
    b2; one DMA stores [4, 4096] = out^T per batch (host transposes).
"""

import numpy as np

B, S, I, H, O = 16, 4096, 64, 64, 2
A = 8                 # atten_size; window = 2*A+1 = 17
NCORES = 8
BPC = B // NCORES     # batches per core = 2
CHUNK = 512           # machine columns per PSUM-stage chunk
GCHUNK = 2048         # seq positions per load group
NG = S // GCHUNK      # 2
JG = GCHUNK // 128    # 16 u-slices per load group
PAD = 4240            # 8 zero pad + 4096 + 136 tail pad

_PROGRAM = None


def _build_program():
    import concourse.bacc as bacc
    import concourse.mybir as mybir
    from concourse.tile import TileContext

    f32 = mybir.dt.float32
    f32r = mybir.dt.float32r

    nc = bacc.Bacc()

    x_d = nc.declare_dram_parameter("x", [BPC, I, S], f32, isOutput=False)
    p_d = nc.declare_dram_parameter("p", [BPC, I, S], f32, isOutput=False)
    c_d = nc.declare_dram_parameter("consts", [128, 262], f32r, isOutput=False)
    o_d = nc.declare_dram_parameter("o", [2 * BPC, S], f32, isOutput=True)

    with TileContext(nc) as tc:
        with (
            tc.tile_pool(name="const", bufs=1) as const,
            tc.tile_pool(name="inp", bufs=3) as inp,
            tc.tile_pool(name="hsb", bufs=4) as hsb,
            tc.tile_pool(name="pp", bufs=1) as pp,
            tc.tile_pool(name="wsum", bufs=2) as wsum,
            tc.tile_pool(name="ps_t", bufs=2, space="PSUM") as ps_t,
            tc.tile_pool(name="ps_h", bufs=3, space="PSUM") as ps_h,
            tc.tile_pool(name="ps_p", bufs=2, space="PSUM") as ps_p,
            tc.tile_pool(name="ps_w", bufs=1, space="PSUM") as ps_w,
        ):
            c_t = const.tile([128, 262], f32r)
            nc.sync.dma_start(out=c_t[:], in_=c_d[:])
            w1_t = c_t[:, 0:128]
            wc_t = c_t[:, 128 : 128 + 2 * BPC]
            id_t = c_t[:, 132:260]                    # [128, 128] f32r identity
            b1_t = c_t[:, 260:261].bitcast(f32)
            b2_t = c_t[:, 261:262].bitcast(f32)

            # Warm up the tensor engine's HAM clock gate while the first
            # loads are in flight, so real matmuls start at 2.4 GHz.
            wps = ps_w.tile([128, 256], f32)
            for _ in range(16):
                nc.tensor.matmul(
                    out=wps[:], lhsT=c_t[:, 0:128], rhs=c_t[:, 0:256],
                    start=True, stop=True,
                )

            # p_pad[(2b+f), 8 + s] = p^T; zero pads at both ends.
            p_pad = pp.tile([2 * BPC, PAD], f32)
            nc.vector.memset(p_pad[:, 0:8], 0.0)
            nc.vector.memset(p_pad[:, 8 + S : PAD], 0.0)

            for g in range(NG):
                s0 = g * GCHUNK
                # Feature-major tiles: partition (b, f) = 64b + f, free = seq.
                # DRAM rows are 16 KB contiguous per partition -> minimal DMA
                # descriptor count; (b f) merges since b-stride = 64*f-stride.
                xt = inp.tile([128, GCHUNK], f32, tag="xt")
                pt = inp.tile([128, GCHUNK], f32, tag="pt")
                bounds = [0, 1024, 2048]
                for lo, hi in zip(bounds[:-1], bounds[1:]):
                    nc.sync.dma_start(
                        out=xt[:, lo:hi],
                        in_=x_d[:, :, s0 + lo : s0 + hi].rearrange(
                            "b f s -> (b f) s"
                        ),
                    )
                    nc.sync.dma_start(
                        out=pt[:, lo:hi],
                        in_=p_d[:, :, s0 + lo : s0 + hi].rearrange(
                            "b f s -> (b f) s"
                        ),
                    )
                # h0T = x + pos, already in the matmul rhs layout; adds split
                # per 512-col chunk between DVE and GpSimd.
                h0T = inp.tile([128, GCHUNK], f32r, tag="h0")
                for kk in range(GCHUNK // CHUNK):
                    cs = slice(CHUNK * kk, CHUNK * (kk + 1))
                    nc.vector.tensor_add(
                        out=h0T[:, cs], in0=xt[:, cs], in1=pt[:, cs]
                    )

                for kk in range(GCHUNK // CHUNK):
                    cs = slice(CHUNK * kk, CHUNK * (kk + 1))
                    hT_ps = ps_h.tile([128, CHUNK], f32)
                    nc.tensor.matmul(
                        out=hT_ps[:], lhsT=w1_t, rhs=h0T[:, cs], start=True,
                        stop=True,
                    )
                    hT = hsb.tile([128, CHUNK], f32r, tag="hT")
                    nc.scalar.activation(
                        out=hT[:],
                        in_=hT_ps[:],
                        func=mybir.ActivationFunctionType.Relu,
                        bias=b1_t,
                    )

                    pT_ps = ps_p.tile([2 * BPC, CHUNK], f32)
                    nc.tensor.matmul(
                        out=pT_ps[:], lhsT=wc_t, rhs=hT[:], start=True, stop=True
                    )
                    # Columns are in natural seq order: contiguous scatter.
                    # The final copies go to DVE (idle by then) so p_pad
                    # completes sooner and unblocks the halo gather.
                    base = 8 + s0 + CHUNK * kk
                    if g == NG - 1 and kk >= 2:
                        nc.vector.tensor_copy(
                            out=p_pad[:, base : base + CHUNK], in_=pT_ps[:]
                        )
                    else:
                        nc.scalar.copy(
                            out=p_pad[:, base : base + CHUNK], in_=pT_ps[:]
                        )

            # Re-partition into halo layout Q[(bf,c), u] = p^T[bf, 128c+u-8]:
            # src iterates (bf, c, u), dest partition = 32*bf + c.
            q_t = wsum.tile([128, 144], f32, tag="q")
            main = p_pad[:, 0 : 128 * 32].rearrange("p (c u) -> p c u", u=128)
            nc.scalar.dma_start(out=q_t[:, 0:64], in_=main[:, :, 0:64])
            nc.sync.dma_start(out=q_t[:, 64:128], in_=main[:, :, 64:128])
            nc.sync.dma_start(
                out=q_t[:, 128:136],
                in_=p_pad[:, 128 : 128 + 128 * 32].rearrange(
                    "p (c u) -> p c u", u=128
                )[:, :, 0:8],
            )
            nc.sync.dma_start(
                out=q_t[:, 136:144],
                in_=p_pad[:, 136 : 136 + 128 * 32].rearrange(
                    "p (c u) -> p c u", u=128
                )[:, :, 0:8],
            )

            # 17-wide window sum via doubling: ws[u] = sum_{d=0..16} Q[u+d].
            t2 = wsum.tile([128, 143], f32, tag="t2")
            nc.vector.tensor_add(out=t2[:], in0=q_t[:, 0:143], in1=q_t[:, 1:144])
            t4 = wsum.tile([128, 141], f32, tag="t4")
            nc.vector.tensor_add(out=t4[:], in0=t2[:, 0:141], in1=t2[:, 2:143])
            t8 = wsum.tile([128, 137], f32, tag="t8")
            nc.vector.tensor_add(out=t8[:], in0=t4[:, 0:137], in1=t4[:, 4:141])
            t16 = wsum.tile([128, 129], f32, tag="t16")
            nc.vector.tensor_add(out=t16[:], in0=t8[:, 0:129], in1=t8[:, 8:137])
            ws_t = wsum.tile([128, 128], f32, tag="ws")
            nc.vector.tensor_add(out=ws_t[:], in0=t16[:, 0:128], in1=q_t[:, 16:144])

            ows = wsum.tile([128, 128], f32, tag="ows")
            nc.scalar.activation(
                out=ows[:],
                in_=ws_t[:],
                func=mybir.ActivationFunctionType.Identity,
                bias=b2_t,
            )
            nc.sync.dma_start(
                out=o_d[:, :].rearrange("p (c u) -> (p c) u", u=128), in_=ows[:]
            )

    nc.finalize()
    return nc


def _get_program():
    global _PROGRAM
    if _PROGRAM is None:
        _PROGRAM = _build_program()
    return _PROGRAM


def _host_inputs(W1, b1, Wv, W2, b2):
    """Pack the small replicated parameters into one [128, 262] f32 tensor."""
    W1 = np.asarray(W1, np.float32).reshape(I, H)
    Wc = (
        np.asarray(Wv, np.float32).reshape(H, H)
        @ np.asarray(W2, np.float32).reshape(H, O)
    ).astype(np.float32)
    consts = np.zeros((128, 262), np.float32)
    consts[:64, 0:64] = W1
    consts[64:, 64:128] = W1
    consts[:64, 128 : 128 + O] = Wc
    consts[64:, 128 + O : 128 + 2 * O] = Wc
    consts[:, 132:260] = np.eye(128, dtype=np.float32)
    consts[:, 260] = np.tile(np.asarray(b1, np.float32).reshape(H), BPC)
    # winsum partition layout is (b, f, c): f = (p % 64) // 32
    pidx = np.arange(128)
    consts[:, 261] = np.asarray(b2, np.float32).reshape(O)[(pidx % 64) // 32]
    return consts


def _in_maps(x, pos_enc, W1, b1, Wv, W2, b2):
    x = np.asarray(x, np.float32)
    pos_enc = np.asarray(pos_enc, np.float32)
    consts = _host_inputs(W1, b1, Wv, W2, b2)
    in_maps = []
    for r in range(NCORES):
        sl = slice(r * BPC, (r + 1) * BPC)
        in_maps.append(
            {
                "x": np.ascontiguousarray(x[sl].transpose(0, 2, 1)),
                "p": np.ascontiguousarray(pos_enc[sl].transpose(0, 2, 1)),
                "consts": consts,
            }
        )
    return in_maps


def _assemble(results):
    out = np.empty((B, S, O), np.float32)
    for r in range(NCORES):
        o = np.asarray(results[r]["o"]).reshape(BPC, O, S)
        out[r * BPC : (r + 1) * BPC] = o.transpose(0, 2, 1)
    weights = np.ones((B, S, 1, 2 * A + 1), np.float32)
    return out, weights


def kernel(x, pos_enc, W1, b1, Wq, Wk, Wv, W2, b2):
    from concourse.bass_utils import run_bass_kernel_spmd

    in_maps = _in_maps(x, pos_enc, W1, b1, Wv, W2, b2)
    nc = _get_program()
    res = run_bass_kernel_spmd(nc, in_maps, list(range(NCORES))).results
    return _assemble(res)


# revision 50
# speedup vs baseline: 1.0291x; 1.0291x over previous
"""Trainium2 Bass kernel for a sliding-window self-attention block.

The reference network applies softmax over a singleton axis, so the attention
weights are identically 1.0 and the whole module reduces to:

    h   = relu((x + pos_enc) @ W1 + b1)            # [B, S, 64]
    p   = h @ (Wv @ W2)                            # [B, S, 2]
    out = sliding_window_sum(p, +-8, zero-pad) + b2
    weights = ones([B, S, 1, 17])

Sharding: data-parallel over batch, 2 batches per core on 8 NeuronCores.

Per-core dataflow (batches b0/b1 processed jointly on 128 partitions):
  - Inputs arrive feature-major ([B, I, S]; the host folds the transpose
    into the shard copy it makes anyway), so partition (b, f) tiles load
    with 16 KB-contiguous rows and no on-chip transpose is needed.
  - DVE adds -> h0^T in f32r, directly the matmul moving operand.
  - PE warmup matmuls run during the load phase to release the HAM clock
    gate before real work arrives.
  - One block-diagonal f32r matmul per 512-col chunk applies W1 to both
    batches; ACT relu(+b1); second block-diag matmul applies Wc = Wv@W2.
  - p^T chunks scatter (un-permuting seq) into a zero-padded [4, 4240]
    buffer; 3 SBUF DMAs re-partition it into a [(b,f,c), 144] halo
    layout; 5 log-tree DVE adds compute the 17-wide window sum; ACT adds
    b2; one DMA stores [4, 4096] = out^T per batch (host transposes).
"""

import numpy as np

B, S, I, H, O = 16, 4096, 64, 64, 2
A = 8                 # atten_size; window = 2*A+1 = 17
NCORES = 8
BPC = B // NCORES     # batches per core = 2
CHUNK = 512           # machine columns per PSUM-stage chunk
GCHUNK = 2048         # seq positions per load group
NG = S // GCHUNK      # 2
JG = GCHUNK // 128    # 16 u-slices per load group
PAD = 4240            # 8 zero pad + 4096 + 136 tail pad

_PROGRAM = None


def _build_program():
    import concourse.bacc as bacc
    import concourse.mybir as mybir
    from concourse.tile import TileContext

    f32 = mybir.dt.float32
    f32r = mybir.dt.float32r

    nc = bacc.Bacc()

    x_d = nc.declare_dram_parameter("x", [BPC, I, S], f32, isOutput=False)
    p_d = nc.declare_dram_parameter("p", [BPC, I, S], f32, isOutput=False)
    c_d = nc.declare_dram_parameter("consts", [128, 262], f32r, isOutput=False)
    o_d = nc.declare_dram_parameter("o", [2 * BPC, S], f32, isOutput=True)

    with TileContext(nc) as tc:
        with (
            tc.tile_pool(name="const", bufs=1) as const,
            tc.tile_pool(name="inp", bufs=3) as inp,
            tc.tile_pool(name="hsb", bufs=4) as hsb,
            tc.tile_pool(name="pp", bufs=1) as pp,
            tc.tile_pool(name="wsum", bufs=2) as wsum,
            tc.tile_pool(name="ps_h", bufs=3, space="PSUM") as ps_h,
            tc.tile_pool(name="ps_p", bufs=2, space="PSUM") as ps_p,
            tc.tile_pool(name="ps_w", bufs=1, space="PSUM") as ps_w,
        ):
            c_t = const.tile([128, 262], f32r)
            nc.sync.dma_start(out=c_t[:], in_=c_d[:])
            w1_t = c_t[:, 0:128]
            wc_t = c_t[:, 128 : 128 + 2 * BPC]
            id_t = c_t[:, 132:260]                    # [128, 128] f32r identity
            b1_t = c_t[:, 260:261].bitcast(f32)
            b2_t = c_t[:, 261:262].bitcast(f32)

            # Warm up the tensor engine's HAM clock gate while the first
            # loads are in flight, so real matmuls start at 2.4 GHz.
            wps = ps_w.tile([128, 256], f32)
            for _ in range(16):
                nc.tensor.matmul(
                    out=wps[:], lhsT=c_t[:, 0:128], rhs=c_t[:, 0:256],
                    start=True, stop=True,
                )

            # p_pad[(2b+f), 8 + s] = p^T; zero pads at both ends.
            p_pad = pp.tile([2 * BPC, PAD], f32)
            nc.vector.memset(p_pad[:, 0:8], 0.0)
            nc.vector.memset(p_pad[:, 8 + S : PAD], 0.0)

            for g in range(NG):
                s0 = g * GCHUNK
                # Feature-major tiles: partition (b, f) = 64b + f, free = seq.
                # DRAM rows are 16 KB contiguous per partition -> minimal DMA
                # descriptor count; (b f) merges since b-stride = 64*f-stride.
                xt = inp.tile([128, GCHUNK], f32, tag="xt")
                pt = inp.tile([128, GCHUNK], f32, tag="pt")
                bounds = [0, 1024, 2048]
                for lo, hi in zip(bounds[:-1], bounds[1:]):
                    nc.sync.dma_start(
                        out=xt[:, lo:hi],
                        in_=x_d[:, :, s0 + lo : s0 + hi].rearrange(
                            "b f s -> (b f) s"
                        ),
                    )
                    nc.sync.dma_start(
                        out=pt[:, lo:hi],
                        in_=p_d[:, :, s0 + lo : s0 + hi].rearrange(
                            "b f s -> (b f) s"
                        ),
                    )
                # h0T = x + pos, already in the matmul rhs layout; adds split
                # per 512-col chunk between DVE and GpSimd.
                h0T = inp.tile([128, GCHUNK], f32r, tag="h0")
                for kk in range(GCHUNK // CHUNK):
                    cs = slice(CHUNK * kk, CHUNK * (kk + 1))
                    nc.vector.tensor_add(
                        out=h0T[:, cs], in0=xt[:, cs], in1=pt[:, cs]
                    )

                for kk in range(GCHUNK // CHUNK):
                    cs = slice(CHUNK * kk, CHUNK * (kk + 1))
                    hT_ps = ps_h.tile([128, CHUNK], f32)
                    nc.tensor.matmul(
                        out=hT_ps[:], lhsT=w1_t, rhs=h0T[:, cs], start=True,
                        stop=True,
                    )
                    hT = hsb.tile([128, CHUNK], f32r, tag="hT")
                    nc.scalar.activation(
                        out=hT[:],
                        in_=hT_ps[:],
                        func=mybir.ActivationFunctionType.Relu,
                        bias=b1_t,
                    )

                    pT_ps = ps_p.tile([2 * BPC, CHUNK], f32)
                    nc.tensor.matmul(
                        out=pT_ps[:], lhsT=wc_t, rhs=hT[:], start=True, stop=True
                    )
                    # Columns are in natural seq order: contiguous scatter.
                    base = 8 + s0 + CHUNK * kk
                    nc.scalar.copy(
                        out=p_pad[:, base : base + CHUNK], in_=pT_ps[:]
                    )

            # Re-partition into halo layout Q[(bf,c), u] = p^T[bf, 128c+u-8]:
            # src iterates (bf, c, u), dest partition = 32*bf + c.
            q_t = wsum.tile([128, 144], f32, tag="q")
            nc.scalar.dma_start(
                out=q_t[:, 0:128],
                in_=p_pad[:, 0 : 128 * 32].rearrange("p (c u) -> p c u", u=128),
            )
            nc.sync.dma_start(
                out=q_t[:, 128:136],
                in_=p_pad[:, 128 : 128 + 128 * 32].rearrange(
                    "p (c u) -> p c u", u=128
                )[:, :, 0:8],
            )
            nc.sync.dma_start(
                out=q_t[:, 136:144],
                in_=p_pad[:, 136 : 136 + 128 * 32].rearrange(
                    "p (c u) -> p c u", u=128
                )[:, :, 0:8],
            )

            # 17-wide window sum via doubling: ws[u] = sum_{d=0..16} Q[u+d].
            t2 = wsum.tile([128, 143], f32, tag="t2")
            nc.vector.tensor_add(out=t2[:], in0=q_t[:, 0:143], in1=q_t[:, 1:144])
            t4 = wsum.tile([128, 141], f32, tag="t4")
            nc.vector.tensor_add(out=t4[:], in0=t2[:, 0:141], in1=t2[:, 2:143])
            t8 = wsum.tile([128, 137], f32, tag="t8")
            nc.vector.tensor_add(out=t8[:], in0=t4[:, 0:137], in1=t4[:, 4:141])
            t16 = wsum.tile([128, 129], f32, tag="t16")
            nc.vector.tensor_add(out=t16[:], in0=t8[:, 0:129], in1=t8[:, 8:137])
            ws_t = wsum.tile([128, 128], f32, tag="ws")
            nc.vector.tensor_add(out=ws_t[:], in0=t16[:, 0:128], in1=q_t[:, 16:144])

            ows = wsum.tile([128, 128], f32, tag="ows")
            nc.scalar.activation(
                out=ows[:],
                in_=ws_t[:],
                func=mybir.ActivationFunctionType.Identity,
                bias=b2_t,
            )
            nc.sync.dma_start(
                out=o_d[:, :].rearrange("p (c u) -> (p c) u", u=128), in_=ows[:]
            )

    nc.finalize()
    return nc


def _get_program():
    global _PROGRAM
    if _PROGRAM is None:
        _PROGRAM = _build_program()
    return _PROGRAM


def _host_inputs(W1, b1, Wv, W2, b2):
    """Pack the small replicated parameters into one [128, 262] f32 tensor."""
    W1 = np.asarray(W1, np.float32).reshape(I, H)
    Wc = (
        np.asarray(Wv, np.float32).reshape(H, H)
        @ np.asarray(W2, np.float32).reshape(H, O)
    ).astype(np.float32)
    consts = np.zeros((128, 262), np.float32)
    consts[:64, 0:64] = W1
    consts[64:, 64:128] = W1
    consts[:64, 128 : 128 + O] = Wc
    consts[64:, 128 + O : 128 + 2 * O] = Wc
    consts[:, 132:260] = np.eye(128, dtype=np.float32)
    consts[:, 260] = np.tile(np.asarray(b1, np.float32).reshape(H), BPC)
    # winsum partition layout is (b, f, c): f = (p % 64) // 32
    pidx = np.arange(128)
    consts[:, 261] = np.asarray(b2, np.float32).reshape(O)[(pidx % 64) // 32]
    return consts


def _in_maps(x, pos_enc, W1, b1, Wv, W2, b2):
    x = np.asarray(x, np.float32)
    pos_enc = np.asarray(pos_enc, np.float32)
    consts = _host_inputs(W1, b1, Wv, W2, b2)
    in_maps = []
    for r in range(NCORES):
        sl = slice(r * BPC, (r + 1) * BPC)
        in_maps.append(
            {
                "x": np.ascontiguousarray(x[sl].transpose(0, 2, 1)),
                "p": np.ascontiguousarray(pos_enc[sl].transpose(0, 2, 1)),
                "consts": consts,
            }
        )
    return in_maps


def _assemble(results):
    out = np.empty((B, S, O), np.float32)
    for r in range(NCORES):
        o = np.asarray(results[r]["o"]).reshape(BPC, O, S)
        out[r * BPC : (r + 1) * BPC] = o.transpose(0, 2, 1)
    weights = np.ones((B, S, 1, 2 * A + 1), np.float32)
    return out, weights


def kernel(x, pos_enc, W1, b1, Wq, Wk, Wv, W2, b2):
    from concourse.bass_utils import run_bass_kernel_spmd

    in_maps = _in_maps(x, pos_enc, W1, b1, Wv, W2, b2)
    nc = _get_program()
    res = run_bass_kernel_spmd(nc, in_maps, list(range(NCORES))).results
    return _assemble(res)


# revision 51
# speedup vs baseline: 1.0297x; 1.0006x over previous
"""Trainium2 Bass kernel for a sliding-window self-attention block.

The reference network applies softmax over a singleton axis, so the attention
weights are identically 1.0 and the whole module reduces to:

    h   = relu((x + pos_enc) @ W1 + b1)            # [B, S, 64]
    p   = h @ (Wv @ W2)                            # [B, S, 2]
    out = sliding_window_sum(p, +-8, zero-pad) + b2
    weights = ones([B, S, 1, 17])

Sharding: data-parallel over batch, 2 batches per core on 8 NeuronCores.

Per-core dataflow (batches b0/b1 processed jointly on 128 partitions):
  - Inputs arrive feature-major ([B, I, S]; the host folds the transpose
    into the shard copy it makes anyway), so partition (b, f) tiles load
    with 16 KB-contiguous rows and no on-chip transpose is needed.
  - DVE adds -> h0^T in f32r, directly the matmul moving operand.
  - PE warmup matmuls run during the load phase to release the HAM clock
    gate before real work arrives.
  - One block-diagonal f32r matmul per 512-col chunk applies W1 to both
    batches; ACT relu(+b1); second block-diag matmul applies Wc = Wv@W2.
  - p^T chunks scatter (un-permuting seq) into a zero-padded [4, 4240]
    buffer; 3 SBUF DMAs re-partition it into a [(b,f,c), 144] halo
    layout; 5 log-tree DVE adds compute the 17-wide window sum; ACT adds
    b2; one DMA stores [4, 4096] = out^T per batch (host transposes).
"""

import numpy as np

B, S, I, H, O = 16, 4096, 64, 64, 2
A = 8                 # atten_size; window = 2*A+1 = 17
NCORES = 8
BPC = B // NCORES     # batches per core = 2
CHUNK = 512           # machine columns per PSUM-stage chunk
GCHUNK = 2048         # seq positions per load group
NG = S // GCHUNK      # 2
JG = GCHUNK // 128    # 16 u-slices per load group
PAD = 4240            # 8 zero pad + 4096 + 136 tail pad

_PROGRAM = None


def _build_program():
    import concourse.bacc as bacc
    import concourse.mybir as mybir
    from concourse.tile import TileContext

    f32 = mybir.dt.float32
    f32r = mybir.dt.float32r

    nc = bacc.Bacc()

    x_d = nc.declare_dram_parameter("x", [BPC, I, S], f32, isOutput=False)
    p_d = nc.declare_dram_parameter("p", [BPC, I, S], f32, isOutput=False)
    c_d = nc.declare_dram_parameter("consts", [128, 262], f32r, isOutput=False)
    o_d = nc.declare_dram_parameter("o", [2 * BPC, S], f32, isOutput=True)

    with TileContext(nc) as tc:
        with (
            tc.tile_pool(name="const", bufs=1) as const,
            tc.tile_pool(name="inp", bufs=3) as inp,
            tc.tile_pool(name="hsb", bufs=4) as hsb,
            tc.tile_pool(name="pp", bufs=1) as pp,
            tc.tile_pool(name="wsum", bufs=2) as wsum,
            tc.tile_pool(name="ps_h", bufs=3, space="PSUM") as ps_h,
            tc.tile_pool(name="ps_p", bufs=2, space="PSUM") as ps_p,
            tc.tile_pool(name="ps_w", bufs=1, space="PSUM") as ps_w,
        ):
            c_t = const.tile([128, 262], f32r)
            nc.sync.dma_start(out=c_t[:], in_=c_d[:])
            w1_t = c_t[:, 0:128]
            wc_t = c_t[:, 128 : 128 + 2 * BPC]
            id_t = c_t[:, 132:260]                    # [128, 128] f32r identity
            b1_t = c_t[:, 260:261].bitcast(f32)
            b2_t = c_t[:, 261:262].bitcast(f32)

            # Warm up the tensor engine's HAM clock gate while the first
            # loads are in flight, so real matmuls start at 2.4 GHz.
            wps = ps_w.tile([128, 256], f32)
            for _ in range(16):
                nc.tensor.matmul(
                    out=wps[:], lhsT=c_t[:, 0:128], rhs=c_t[:, 0:256],
                    start=True, stop=True,
                )

            # p_pad[(2b+f), 8 + s] = p^T; zero pads at both ends.
            p_pad = pp.tile([2 * BPC, PAD], f32)
            nc.vector.memset(p_pad[:, 0:8], 0.0)
            nc.vector.memset(p_pad[:, 8 + S : PAD], 0.0)

            for g in range(NG):
                s0 = g * GCHUNK
                # Feature-major tiles: partition (b, f) = 64b + f, free = seq.
                # DRAM rows are 16 KB contiguous per partition -> minimal DMA
                # descriptor count; (b f) merges since b-stride = 64*f-stride.
                xt = inp.tile([128, GCHUNK], f32, tag="xt")
                pt = inp.tile([128, GCHUNK], f32, tag="pt")
                bounds = [0, 1024, 2048]
                for lo, hi in zip(bounds[:-1], bounds[1:]):
                    nc.sync.dma_start(
                        out=xt[:, lo:hi],
                        in_=x_d[:, :, s0 + lo : s0 + hi].rearrange(
                            "b f s -> (b f) s"
                        ),
                    )
                    nc.sync.dma_start(
                        out=pt[:, lo:hi],
                        in_=p_d[:, :, s0 + lo : s0 + hi].rearrange(
                            "b f s -> (b f) s"
                        ),
                    )
                # h0T = x + pos, already in the matmul rhs layout; adds split
                # per 512-col chunk between DVE and GpSimd.
                h0T = inp.tile([128, GCHUNK], f32r, tag="h0")
                for kk in range(GCHUNK // CHUNK):
                    cs = slice(CHUNK * kk, CHUNK * (kk + 1))
                    nc.vector.tensor_add(
                        out=h0T[:, cs], in0=xt[:, cs], in1=pt[:, cs]
                    )

                for kk in range(GCHUNK // CHUNK):
                    cs = slice(CHUNK * kk, CHUNK * (kk + 1))
                    hT_ps = ps_h.tile([128, CHUNK], f32)
                    nc.tensor.matmul(
                        out=hT_ps[:], lhsT=w1_t, rhs=h0T[:, cs], start=True,
                        stop=True,
                    )
                    hT = hsb.tile([128, CHUNK], f32r, tag="hT")
                    nc.scalar.activation(
                        out=hT[:],
                        in_=hT_ps[:],
                        func=mybir.ActivationFunctionType.Relu,
                        bias=b1_t,
                    )

                    pT_ps = ps_p.tile([2 * BPC, CHUNK], f32)
                    nc.tensor.matmul(
                        out=pT_ps[:], lhsT=wc_t, rhs=hT[:], start=True, stop=True
                    )
                    # Columns are in natural seq order: contiguous scatter.
                    base = 8 + s0 + CHUNK * kk
                    nc.scalar.copy(
                        out=p_pad[:, base : base + CHUNK], in_=pT_ps[:]
                    )

            # Re-partition into halo layout Q[(bf,c), u] = p^T[bf, 128c+u-8]:
            # src iterates (bf, c, u), dest partition = 32*bf + c.
            q_t = wsum.tile([128, 144], f32, tag="q")
            # Main band split by c-range: c<16 depends only on the first 2048
            # seq (ready mid-kernel, transfers overlap g=1 compute on the idle
            # SP queue); only c>=16 remains for the tail.
            for m in range(2 * BPC):
                nc.sync.dma_start(
                    out=q_t[32 * m : 32 * m + 16, 0:128],
                    in_=p_pad[m : m + 1, 0:2048].rearrange(
                        "p (c u) -> p c u", u=128
                    ),
                )
            for m in range(2 * BPC):
                nc.sync.dma_start(
                    out=q_t[32 * m + 16 : 32 * m + 32, 0:128],
                    in_=p_pad[m : m + 1, 2048:4096].rearrange(
                        "p (c u) -> p c u", u=128
                    ),
                )
            nc.sync.dma_start(
                out=q_t[:, 128:136],
                in_=p_pad[:, 128 : 128 + 128 * 32].rearrange(
                    "p (c u) -> p c u", u=128
                )[:, :, 0:8],
            )
            nc.sync.dma_start(
                out=q_t[:, 136:144],
                in_=p_pad[:, 136 : 136 + 128 * 32].rearrange(
                    "p (c u) -> p c u", u=128
                )[:, :, 0:8],
            )

            # 17-wide window sum via doubling: ws[u] = sum_{d=0..16} Q[u+d].
            t2 = wsum.tile([128, 143], f32, tag="t2")
            nc.vector.tensor_add(out=t2[:], in0=q_t[:, 0:143], in1=q_t[:, 1:144])
            t4 = wsum.tile([128, 141], f32, tag="t4")
            nc.vector.tensor_add(out=t4[:], in0=t2[:, 0:141], in1=t2[:, 2:143])
            t8 = wsum.tile([128, 137], f32, tag="t8")
            nc.vector.tensor_add(out=t8[:], in0=t4[:, 0:137], in1=t4[:, 4:141])
            t16 = wsum.tile([128, 129], f32, tag="t16")
            nc.vector.tensor_add(out=t16[:], in0=t8[:, 0:129], in1=t8[:, 8:137])
            ws_t = wsum.tile([128, 128], f32, tag="ws")
            nc.vector.tensor_add(out=ws_t[:], in0=t16[:, 0:128], in1=q_t[:, 16:144])

            ows = wsum.tile([128, 128], f32, tag="ows")
            nc.scalar.activation(
                out=ows[:],
                in_=ws_t[:],
                func=mybir.ActivationFunctionType.Identity,
                bias=b2_t,
            )
            nc.sync.dma_start(
                out=o_d[:, :].rearrange("p (c u) -> (p c) u", u=128), in_=ows[:]
            )

    nc.finalize()
    return nc


def _get_program():
    global _PROGRAM
    if _PROGRAM is None:
        _PROGRAM = _build_program()
    return _PROGRAM


def _host_inputs(W1, b1, Wv, W2, b2):
    """Pack the small replicated parameters into one [128, 262] f32 tensor."""
    W1 = np.asarray(W1, np.float32).reshape(I, H)
    Wc = (
        np.asarray(Wv, np.float32).reshape(H, H)
        @ np.asarray(W2, np.float32).reshape(H, O)
    ).astype(np.float32)
    consts = np.zeros((128, 262), np.float32)
    consts[:64, 0:64] = W1
    consts[64:, 64:128] = W1
    consts[:64, 128 : 128 + O] = Wc
    consts[64:, 128 + O : 128 + 2 * O] = Wc
    consts[:, 132:260] = np.eye(128, dtype=np.float32)
    consts[:, 260] = np.tile(np.asarray(b1, np.float32).reshape(H), BPC)
    # winsum partition layout is (b, f, c): f = (p % 64) // 32
    pidx = np.arange(128)
    consts[:, 261] = np.asarray(b2, np.float32).reshape(O)[(pidx % 64) // 32]
    return consts


def _in_maps(x, pos_enc, W1, b1, Wv, W2, b2):
    x = np.asarray(x, np.float32)
    pos_enc = np.asarray(pos_enc, np.float32)
    consts = _host_inputs(W1, b1, Wv, W2, b2)
    in_maps = []
    for r in range(NCORES):
        sl = slice(r * BPC, (r + 1) * BPC)
        in_maps.append(
            {
                "x": np.ascontiguousarray(x[sl].transpose(0, 2, 1)),
                "p": np.ascontiguousarray(pos_enc[sl].transpose(0, 2, 1)),
                "consts": consts,
            }
        )
    return in_maps


def _assemble(results):
    out = np.empty((B, S, O), np.float32)
    for r in range(NCORES):
        o = np.asarray(results[r]["o"]).reshape(BPC, O, S)
        out[r * BPC : (r + 1) * BPC] = o.transpose(0, 2, 1)
    weights = np.ones((B, S, 1, 2 * A + 1), np.float32)
    return out, weights


def kernel(x, pos_enc, W1, b1, Wq, Wk, Wv, W2, b2):
    from concourse.bass_utils import run_bass_kernel_spmd

    in_maps = _in_maps(x, pos_enc, W1, b1, Wv, W2, b2)
    nc = _get_program()
    res = run_bass_kernel_spmd(nc, in_maps, list(range(NCORES))).results
    return _assemble(res)
